# revision 18
# baseline (speedup 1.0000x reference)
"""PiNet2 GNN message-passing kernel for 8 Trainium2 NeuronCores.

kernel(**inputs) takes FULL unsharded numpy inputs (as in setup_inputs) and
returns the full (p1t1, p3t1) tuple.

Design (edge parallelism per sharding hint; pairs sharded, atoms replicated):
  phase A (per-atom, replicated on each core):
    p1h = tanh(tanh(p1@Wpp1a+b1a)@Wpp1b+b1b)
    A   = p1h@Wpi1[:C] + bpi1          (stored as 256B DRAM rows, bf16-padded)
    B   = p1h@Wpi1[C:]                 packed with P3J = p3@(Wpp3a@Wpp3b@Wpxj)
                                       as [B|P3J] 512B bf16 DRAM rows
    P3I = p3@(Wpp3a@Wpp3b@Wpxi)        kept channels-major for phase C
  phase B (per-pair, 80000 pairs/core, sorted by center atom i and bin-packed
  into 128-pair tiles with <=64 distinct atoms each, every atom's pairs in
  exactly one tile):
    dma_gather A[i], [B|P3J][j]
    u = tanh(A[i]+B[j]); v = tanh(u@Wpi2+bpi2) (augmented-ones matmul bias)
    i1 = sum_b v_b * basis_b ; w = tanh(i1@Wii1); t = tanh(w@Wii2)
    crossJD = P3J[j]*t3 + diff (x) t1
    per-tile segment sum of [t2|t3|crossJD] via membership matmul
    (mem[p,a] = (localidx[p]==a), built by DVE is_equal against an iota tile)
    race-free dma_scatter_add of per-atom partial rows into acc (pre-zeroed
    by declaring it an output -> donated zero buffer)
  ReduceScatter(acc) over the 8 cores; phase C on each core's atom shard:
    p3n = P3I*S3 + cross; dot = sum_x (p3n@Wdi)*(p3n@Wdj)
    p1t1 = dot + t2sum; p3t1 = p3n*p1t1     (outputs transposed; host fixes)
  host concatenates the 8 shards.
"""

import sys
import numpy as np

sys.path.insert(0, "/opt/trn_rl_repo")

# ---------------- problem constants ----------------
N = 20000
P = 640000
C = 64
NB = 4
NCORES = 8
PC = P // NCORES
NPAD = 20480            # padded atom count (40 * 512)
TILE = 128              # pairs per tile
MAXA = 64               # max distinct atoms per tile
T = 672                 # tiles per core (asserted during packing)
TRASH = N               # scatter trash row for pad slots
GCH = 32                # tiles per gather chunk (4096 idxs)
SCH = 16                # tiles per scatter chunk (1024 scatter idxs)
GRP = 4                 # tiles per compute group (512 pairs)
ACH = 512               # atoms per phase-A chunk
RSROWS = NPAD // NCORES
PAYW = 320              # payload: t2(64) | t3(64) | crossJD(192)
ACCW = 384              # acc row width (bf16 768B, mult of 256B)
AW = 128                # A row elems (bf16; 64 data + 64 pad = 256B)
BPJW = 256              # B|P3J row elems (bf16 = 512B)

_CACHE = {}
_USE_DEVICE_REDUCE = True


def _wrap16(a):
    """int16 indices -> wrapped layout [128, n/16] (16-row block replicated 8x,
    one replica per Q7 core, per the dma_gather contract)."""
    a = np.asarray(a, np.int16)
    assert a.size % 16 == 0
    w = np.ascontiguousarray(a.reshape(-1, 16).T)
    return np.ascontiguousarray(np.tile(w, (8, 1)))


def _pack_core(i_arr, j_arr):
    order = np.argsort(i_arr, kind="stable")
    i_s = i_arr[order]
    j_s = j_arr[order]
    uniq, counts = np.unique(i_s, return_counts=True)
    nruns = len(uniq)
    run_tile = np.empty(nruns, np.int64)
    run_local = np.empty(nruns, np.int64)
    run_off = np.empty(nruns, np.int64)
    ct = cp = ca = 0
    cl = counts.tolist()
    for r in range(nruns):
        cnt = cl[r]
        assert cnt <= TILE, f"atom degree {cnt} > {TILE}"
        if cp + cnt > TILE or ca == MAXA:
            ct += 1
            cp = 0
            ca = 0
        run_tile[r] = ct
        run_local[r] = ca
        run_off[r] = cp
        cp += cnt
        ca += 1
    assert ct + 1 <= T, f"need {ct+1} tiles > T={T}"

    nslots = T * TILE
    slot_i = np.zeros(nslots, np.int64)
    slot_j = np.zeros(nslots, np.int64)
    slot_local = np.full(nslots, -1.0, np.float32)
    slot_perm = np.full(nslots, -1, np.int64)
    run_start = np.concatenate([[0], np.cumsum(counts)[:-1]])
    dest = (np.repeat(run_tile * TILE + run_off, counts)
            + (np.arange(len(i_s)) - np.repeat(run_start, counts)))
    slot_i[dest] = i_s
    slot_j[dest] = j_s
    slot_local[dest] = np.repeat(run_local, counts).astype(np.float32)
    slot_perm[dest] = order
    sat = np.full((T, MAXA), TRASH, np.int64)
    sat[run_tile, run_local] = uniq
    return slot_i, slot_j, slot_local, slot_perm, sat


def _preprocess(inputs):
    ind = np.asarray(inputs["ind_2"])
    diff = np.asarray(inputs["diff"], np.float32)
    basis = np.asarray(inputs["basis"], np.float32)

    W3 = np.asarray(inputs["Wpp3a"], np.float32) @ np.asarray(inputs["Wpp3b"], np.float32)
    WJ3 = np.ascontiguousarray(W3 @ np.asarray(inputs["Wpxj"], np.float32))
    WI3 = np.ascontiguousarray(W3 @ np.asarray(inputs["Wpxi"], np.float32))
    Wpi2 = np.asarray(inputs["Wpi2"], np.float32)
    bpi2 = np.asarray(inputs["bpi2"], np.float32)
    perm = np.array([c * NB + b for b in range(NB) for c in range(C)])
    Wpi2aug = np.ascontiguousarray(
        np.vstack([Wpi2[:, perm], bpi2[perm][None, :]]))          # [65, 256]
    Wpi1 = np.asarray(inputs["Wpi1"], np.float32)
    bpi1 = np.asarray(inputs["bpi1"], np.float32)
    Wpi1i = np.ascontiguousarray(np.vstack([Wpi1[:C], bpi1[None, :]]))
    Wpi1j = np.ascontiguousarray(np.vstack([Wpi1[C:], np.zeros((1, C), np.float32)]))

    p1T = np.zeros((C, NPAD), np.float32)
    p1T[:, :N] = np.asarray(inputs["p1"], np.float32).T
    p3T = np.zeros((3, C, NPAD), np.float32)
    p3T[:, :, :N] = np.asarray(inputs["p3"], np.float32).transpose(1, 2, 0)

    shared = {
        "p1T": p1T, "p3T": p3T,
        "Wpp1a": np.ascontiguousarray(inputs["Wpp1a"], dtype=np.float32),
        "bpp1a": np.ascontiguousarray(np.asarray(inputs["bpp1a"], np.float32).reshape(C, 1)),
        "Wpp1b": np.ascontiguousarray(inputs["Wpp1b"], dtype=np.float32),
        "bpp1b": np.ascontiguousarray(np.asarray(inputs["bpp1b"], np.float32).reshape(C, 1)),
        "Wpi1i": Wpi1i, "Wpi1j": Wpi1j, "Wpi2aug": Wpi2aug,
        "Wii1": np.ascontiguousarray(inputs["Wii1"], dtype=np.float32),
        "Wii2": np.ascontiguousarray(inputs["Wii2"], dtype=np.float32),
        "WJ3": WJ3, "WI3": WI3,
        "Wdi": np.ascontiguousarray(inputs["Wdi"], dtype=np.float32),
        "Wdj": np.ascontiguousarray(inputs["Wdj"], dtype=np.float32),
        "iota": np.ascontiguousarray(
            np.broadcast_to(np.arange(MAXA, dtype=np.float32), (128, MAXA))),
        "ident": np.eye(128, dtype=np.float32),
    }

    P3I = (np.asarray(inputs["p3"], np.float32).reshape(-1, C) @ WI3).reshape(N, 3, C)
    P3Ip = np.zeros((NPAD, 3, C), np.float32)
    P3Ip[:N] = P3I
    in_maps = []
    for cid in range(NCORES):
        sl = slice(cid * PC, (cid + 1) * PC)
        si, sj, slocal, sperm, sat = _pack_core(
            ind[sl, 0].astype(np.int64), ind[sl, 1].astype(np.int64))
        d = np.zeros((T * TILE, 4), np.float32)
        b = np.zeros((T * TILE, 4), np.float32)
        valid = sperm >= 0
        d[valid, 0:3] = diff[sl][sperm[valid]]
        b[valid, :] = basis[sl][sperm[valid]]
        m = dict(shared)
        m.update({
            "p3ish": np.ascontiguousarray(
                P3Ip[cid * RSROWS:(cid + 1) * RSROWS].transpose(2, 1, 0)),
            "giw": _wrap16(si),
            "gjw": _wrap16(sj),
            "satw": _wrap16(sat.reshape(-1)),
            "diffh": np.ascontiguousarray(d.reshape(T, TILE, 4).transpose(1, 0, 2)),
            "basish": np.ascontiguousarray(b.reshape(T, TILE, 4).transpose(1, 0, 2)),
            "locf": np.ascontiguousarray(slocal.reshape(T, TILE).T),
        })
        in_maps.append(m)
    return in_maps


def _build_nc():
    import concourse.bacc as bacc
    import concourse.bass as bass
    import concourse.tile as tile
    from concourse import library_config, mybir
    AluOp = mybir.AluOpType
    Act = mybir.ActivationFunctionType
    MS = bass.MemorySpace
    F32 = mybir.dt.float32
    BF16 = mybir.dt.bfloat16
    I16 = mybir.dt.int16

    nc = bacc.Bacc(None, target_bir_lowering=False, debug=False)

    p1T_d = nc.declare_dram_parameter("p1T", [C, NPAD], F32, isOutput=False)
    p3T_d = nc.declare_dram_parameter("p3T", [3, C, NPAD], F32, isOutput=False)
    wnames = {}
    for nm, shp in [("Wpp1a", [C, C]), ("bpp1a", [C, 1]),
                    ("Wpp1b", [C, C]), ("bpp1b", [C, 1]),
                    ("Wpi1i", [C + 1, C]), ("Wpi1j", [C + 1, C]),
                    ("Wpi2aug", [C + 1, C * NB]),
                    ("Wii1", [C, C]), ("Wii2", [C, 3 * C]),
                    ("WJ3", [C, C]), ("WI3", [C, C]),
                    ("Wdi", [C, C]), ("Wdj", [C, C]),
                    ("iota", [128, MAXA]), ("ident", [128, 128])]:
        wnames[nm] = nc.declare_dram_parameter(nm, shp, F32, isOutput=False)
    giw_d = nc.declare_dram_parameter("giw", [128, T * TILE // 16], I16, isOutput=False)
    gjw_d = nc.declare_dram_parameter("gjw", [128, T * TILE // 16], I16, isOutput=False)
    satw_d = nc.declare_dram_parameter("satw", [128, T * MAXA // 16], I16, isOutput=False)
    diffh_d = nc.declare_dram_parameter("diffh", [128, T, 4], F32, isOutput=False)
    basish_d = nc.declare_dram_parameter("basish", [128, T, 4], F32, isOutput=False)
    locf_d = nc.declare_dram_parameter("locf", [128, T], F32, isOutput=False)

    p3ish_d = nc.declare_dram_parameter("p3ish", [C, 3, RSROWS], F32, isOutput=False)
    p1t1T_d = nc.declare_dram_parameter("p1t1T", [C, RSROWS], F32, isOutput=True)
    p3t1T_d = nc.declare_dram_parameter("p3t1T", [C, 3, RSROWS], F32, isOutput=True)
    acc_d = nc.dram_tensor("acc", [NPAD, ACCW], BF16)
    A_d = nc.dram_tensor("A_rows", [NPAD, AW], BF16)
    BPJ_d = nc.dram_tensor("BPJ_rows", [NPAD, BPJW], BF16)
    rs_d = nc.dram_tensor("rs_out", [RSROWS, ACCW], BF16)

    with tile.TileContext(nc) as tc:
        nc.gpsimd.load_library(library_config.mlp)
        with tc.tile_pool(name="wpool", bufs=1) as wp:
            W = {}
            for nm in wnames:
                shp = list(wnames[nm].shape)
                W[nm] = wp.tile(shp, F32, tag=nm, name="W_" + nm)
                nc.sync.dma_start(W[nm][:], wnames[nm][:])
            zacc = wp.tile([128, 16, ACCW], BF16, name="zacc")
            nc.vector.memset(zacc[:], 0.0)
            accz = acc_d[:].rearrange("(k p) c -> p k c", p=128)
            for zk in range(NPAD // 128 // 16):
                nc.sync.dma_start(accz[:, zk * 16:(zk + 1) * 16, :], zacc[:])
            Wb = {}
            for nm in ["Wpi2aug", "Wii1", "Wii2", "ident"]:
                shp = list(wnames[nm].shape)
                Wb[nm] = wp.tile(shp, BF16, tag=nm + "b", name="Wb_" + nm)
                nc.vector.tensor_copy(Wb[nm][:], W[nm][:])

            # ================= PHASE A =================
            with (
                tc.tile_pool(name="pa_sb", bufs=2) as pa,
                tc.tile_pool(name="pa_ps", bufs=2, space=MS.PSUM) as pap,
            ):
                for ch in range(NPAD // ACH):
                    s0 = ch * ACH
                    x1 = pa.tile([C, ACH], F32, tag="x1")
                    nc.sync.dma_start(x1[:], p1T_d[:, s0:s0 + ACH])
                    ph = pap.tile([C, ACH], F32, tag="ph")
                    nc.tensor.matmul(ph[:], W["Wpp1a"][:], x1[:])
                    h1 = pa.tile([C, ACH], F32, tag="h1")
                    nc.scalar.activation(h1[:], ph[:], Act.Tanh, bias=W["bpp1a"][:, 0:1])
                    ph2 = pap.tile([C, ACH], F32, tag="ph")
                    nc.tensor.matmul(ph2[:], W["Wpp1b"][:], h1[:])
                    p1h = pa.tile([C + 1, ACH], F32, tag="p1h")
                    nc.scalar.activation(p1h[0:C, :], ph2[:], Act.Tanh, bias=W["bpp1b"][:, 0:1])
                    nc.vector.memset(p1h[C:C + 1, :], 1.0)

                    pA = pap.tile([128, (ACH // 128) * C], F32, tag="pAB")
                    pB = pap.tile([128, (ACH // 128) * C], F32, tag="pAB")
                    for s4 in range(ACH // 128):
                        lhs = p1h[:, s4 * 128:(s4 + 1) * 128]
                        nc.tensor.matmul(pA[:, s4 * C:(s4 + 1) * C], lhs, W["Wpi1i"][:])
                        nc.tensor.matmul(pB[:, s4 * C:(s4 + 1) * C], lhs, W["Wpi1j"][:])
                    Ast = pa.tile([128, ACH // 128, AW], BF16, tag="Ast")
                    nc.vector.tensor_copy(
                        Ast[:, :, 0:C], pA[:].rearrange("p (s c) -> p s c", c=C))
                    BPJst = pa.tile([128, ACH // 128, BPJW], BF16, tag="BPJst")
                    nc.scalar.activation(
                        BPJst[:, :, 0:C],
                        pB[:].rearrange("p (s c) -> p s c", c=C), Act.Copy)

                    for x in range(3):
                        px = pa.tile([C, ACH], F32, tag="px")
                        nc.sync.dma_start(px[:], p3T_d[x, :, s0:s0 + ACH])
                        pJ = pap.tile([128, (ACH // 128) * C], F32, tag="pJ")
                        for s4 in range(ACH // 128):
                            nc.tensor.matmul(
                                pJ[:, s4 * C:(s4 + 1) * C],
                                px[:, s4 * 128:(s4 + 1) * 128], W["WJ3"][:])
                        if x % 2 == 0:
                            nc.vector.tensor_copy(
                                BPJst[:, :, C + x * C:C + (x + 1) * C],
                                pJ[:].rearrange("p (s c) -> p s c", c=C))
                        else:
                            nc.scalar.activation(
                                BPJst[:, :, C + x * C:C + (x + 1) * C],
                                pJ[:].rearrange("p (s c) -> p s c", c=C), Act.Copy)

                    nc.sync.dma_start(
                        A_d[s0:s0 + ACH, :].rearrange("(s p) c -> p s c", p=128), Ast[:])
                    nc.sync.dma_start(
                        BPJ_d[s0:s0 + ACH, :].rearrange("(s p) c -> p s c", p=128), BPJst[:])

            # ================= PHASE B =================
            with (
                tc.tile_pool(name="pb_g", bufs=2) as pg,
                tc.tile_pool(name="pb_sb", bufs=2) as pb,
                tc.tile_pool(name="pb_scat", bufs=2) as psc,
                tc.tile_pool(name="ps_uw", bufs=2, space=MS.PSUM) as ps_uw,
                tc.tile_pool(name="ps_v", bufs=2, space=MS.PSUM) as ps_v,
                tc.tile_pool(name="ps_i1", bufs=1, space=MS.PSUM) as ps_i1,
                tc.tile_pool(name="ps_t", bufs=2, space=MS.PSUM) as ps_t,
                tc.tile_pool(name="ps_s", bufs=1, space=MS.PSUM) as ps_s,
            ):
                for gc in range(T // GCH):
                    t0g = gc * GCH
                    Ag = pg.tile([128, GCH, AW], BF16, tag="Ag")
                    BPJg = pg.tile([128, GCH, BPJW], BF16, tag="BPJg")
                    gi_sb = pg.tile([128, GCH * TILE // 16], I16, tag="gi")
                    gj_sb = pg.tile([128, GCH * TILE // 16], I16, tag="gj")
                    nc.sync.dma_start(gi_sb[:], giw_d[:, t0g * 8:(t0g + GCH) * 8])
                    nc.sync.dma_start(gj_sb[:], gjw_d[:, t0g * 8:(t0g + GCH) * 8])
                    USE_GATHER = True
                    if USE_GATHER:
                        nc.gpsimd.dma_gather(Ag[:], A_d[:], gi_sb[:],
                                             GCH * TILE, GCH * TILE, AW,
                                             single_packet=False)
                        nc.gpsimd.dma_gather(BPJg[:], BPJ_d[:], gj_sb[:],
                                             GCH * TILE, GCH * TILE, BPJW,
                                             single_packet=False)
                    else:
                        nc.sync.dma_start(
                            Ag[:], A_d[0:GCH * TILE, :].rearrange(
                                "(s p) c -> p s c", p=128))
                        nc.sync.dma_start(
                            BPJg[:], BPJ_d[0:GCH * TILE, :].rearrange(
                                "(s p) c -> p s c", p=128))
                    sat_sb = pg.tile([128, GCH * MAXA // 16], I16, tag="sat")
                    nc.sync.dma_start(sat_sb[:], satw_d[:, t0g * 4:(t0g + GCH) * 4])
                    dloc = pg.tile([128, GCH], F32, tag="dloc")
                    nc.sync.dma_start(dloc[:], locf_d[:, t0g:t0g + GCH])
                    ddif = pg.tile([128, GCH, 4], F32, tag="ddif")
                    nc.sync.dma_start(ddif[:], diffh_d[:, t0g:t0g + GCH, :])
                    dbas = pg.tile([128, GCH, 4], F32, tag="dbas")
                    nc.sync.dma_start(dbas[:], basish_d[:, t0g:t0g + GCH, :])

                    for sc in range(GCH // SCH):
                        scat = psc.tile([128, SCH // 2, ACCW], BF16, tag="scat")
                        nc.vector.memset(scat[:, :, PAYW:ACCW], 0.0)
                        for g2 in range(SCH // GRP):
                            ti = sc * SCH + g2 * GRP
                            tg = t0g + ti
                            # ---- u ----
                            upre = pb.tile([128, GRP, C], F32, tag="upre")
                            nc.vector.tensor_tensor(
                                upre[:], Ag[:, ti:ti + GRP, 0:C],
                                BPJg[:, ti:ti + GRP, 0:C], AluOp.add)
                            puw = ps_uw.tile([128, GRP * 128], F32, tag="puw")
                            for k in range(GRP):
                                nc.tensor.transpose(
                                    puw[0:C, k * 128:(k + 1) * 128],
                                    upre[:, k, :], W["ident"][:])
                            ust = pb.tile([C + 1, GRP * 128], BF16, tag="ust")
                            nc.scalar.activation(ust[0:C, :], puw[0:C, :], Act.Tanh)
                            nc.vector.memset(ust[C:C + 1, :], 1.0)
                            # ---- v ----
                            pv0 = ps_v.tile([128, 512], F32, tag="pv")
                            pv1 = ps_v.tile([128, 512], F32, tag="pv")
                            for k in range(GRP):
                                pv = pv0 if k < 2 else pv1
                                nc.tensor.matmul(
                                    pv[:, (k % 2) * 256:(k % 2 + 1) * 256],
                                    ust[:, k * 128:(k + 1) * 128], Wb["Wpi2aug"][:])
                            vt = pb.tile([128, GRP, C * NB], BF16, tag="vt")
                            nc.scalar.activation(
                                vt[:, 0:2, :].rearrange("p s c -> p (s c)"),
                                pv0[:], Act.Tanh)
                            nc.scalar.activation(
                                vt[:, 2:4, :].rearrange("p s c -> p (s c)"),
                                pv1[:], Act.Tanh)
                            # ---- i1 = sum_b vt_b * basis_b ----
                            basg = dbas[:, ti:ti + GRP, :]
                            qa = pb.tile([128, GRP, C], BF16, tag="qa")
                            qb = pb.tile([128, GRP, C], BF16, tag="qb")
                            s01 = pb.tile([128, GRP, C], BF16, tag="s01")
                            s23 = pb.tile([128, GRP, C], BF16, tag="s23")
                            i1 = pb.tile([128, GRP, C], BF16, tag="i1")
                            nc.vector.tensor_tensor(
                                qa[:], vt[:, :, 0 * C:1 * C],
                                basg[:, :, 0:1].broadcast_to([128, GRP, C]), AluOp.mult)
                            nc.vector.tensor_tensor(
                                qb[:], vt[:, :, 1 * C:2 * C],
                                basg[:, :, 1:2].broadcast_to([128, GRP, C]), AluOp.mult)
                            nc.vector.tensor_tensor(s01[:], qa[:], qb[:], AluOp.add)
                            nc.vector.tensor_tensor(
                                qa[:], vt[:, :, 2 * C:3 * C],
                                basg[:, :, 2:3].broadcast_to([128, GRP, C]), AluOp.mult)
                            nc.vector.tensor_tensor(
                                qb[:], vt[:, :, 3 * C:4 * C],
                                basg[:, :, 3:4].broadcast_to([128, GRP, C]), AluOp.mult)
                            nc.vector.tensor_tensor(s23[:], qa[:], qb[:], AluOp.add)
                            nc.vector.tensor_tensor(i1[:], s01[:], s23[:], AluOp.add)
                            # ---- i1T via PE transpose (f32 path) ----
                            i1f = pb.tile([128, GRP, C], F32, tag="i1f")
                            nc.scalar.activation(i1f[:], i1[:], Act.Copy)
                            pi1 = ps_i1.tile([C, GRP * 128], F32, tag="pi1")
                            for k in range(GRP):
                                nc.tensor.transpose(
                                    pi1[:, k * 128:(k + 1) * 128],
                                    i1f[:, k, :], W["ident"][:])
                            i1T = pb.tile([C, GRP * 128], BF16, tag="i1T")
                            nc.scalar.activation(i1T[:], pi1[:], Act.Copy)
                            # ---- w ---- (reuses rows 64:128 of the puw bank)
                            nc.tensor.matmul(puw[C:2 * C, :], Wb["Wii1"][:], i1T[:])
                            w_t = pb.tile([C, GRP * 128], BF16, tag="w_t")
                            nc.scalar.activation(w_t[:], puw[C:2 * C, :], Act.Tanh)
                            # ---- t ----
                            pt0 = ps_t.tile([128, 2 * 192], F32, tag="pt")
                            pt1 = ps_t.tile([128, 2 * 192], F32, tag="pt")
                            for k in range(GRP):
                                pt = pt0 if k < 2 else pt1
                                nc.tensor.matmul(
                                    pt[:, (k % 2) * 192:(k % 2 + 1) * 192],
                                    w_t[:, k * 128:(k + 1) * 128], Wb["Wii2"][:])
                            tT = pb.tile([128, GRP, 192], BF16, tag="tT")
                            nc.scalar.activation(
                                tT[:, 0:2, :].rearrange("p s c -> p (s c)"),
                                pt0[:], Act.Tanh)
                            nc.scalar.activation(
                                tT[:, 2:4, :].rearrange("p s c -> p (s c)"),
                                pt1[:], Act.Tanh)
                            # ---- crossJD = P3J*t3 + diff (x) t1 ----
                            crs = pb.tile([128, GRP, 192], BF16, tag="crs")
                            tmp2 = pb.tile([128, GRP, 192], BF16, tag="tmp2")
                            t3bc = tT[:, :, 128:192].rearrange(
                                "p s c -> p s () c").broadcast_to([128, GRP, 3, C])
                            t1bc = tT[:, :, 0:64].rearrange(
                                "p s c -> p s () c").broadcast_to([128, GRP, 3, C])
                            difbc = ddif[:, ti:ti + GRP, 0:3].rearrange(
                                "p s x -> p s x ()").broadcast_to([128, GRP, 3, C])
                            nc.vector.tensor_tensor(
                                crs[:].rearrange("p s (x c) -> p s x c", x=3),
                                BPJg[:, ti:ti + GRP, C:BPJW].rearrange(
                                    "p s (x c) -> p s x c", x=3),
                                t3bc, AluOp.mult)
                            nc.vector.tensor_tensor(
                                tmp2[:].rearrange("p s (x c) -> p s x c", x=3),
                                difbc, t1bc, AluOp.mult)
                            nc.vector.tensor_tensor(crs[:], crs[:], tmp2[:], AluOp.add)
                            # ---- membership + scatter matmuls ----
                            mem = pb.tile([128, GRP, MAXA], BF16, tag="mem")
                            for k in range(GRP):
                                nc.vector.tensor_scalar(
                                    mem[:, k, :], W["iota"][:],
                                    dloc[:, ti + k:ti + k + 1], None,
                                    AluOp.is_equal)
                            for k2 in range(GRP // 2):
                                pscat = ps_s.tile([128, PAYW], F32, tag="pscat")
                                for kk in range(2):
                                    k = k2 * 2 + kk
                                    po = kk * MAXA
                                    nc.tensor.matmul(
                                        pscat[po:po + MAXA, 0:128],
                                        mem[:, k, :], tT[:, k, 64:192])
                                    nc.tensor.matmul(
                                        pscat[po:po + MAXA, 128:PAYW],
                                        mem[:, k, :], crs[:, k, :])
                                col = g2 * (GRP // 2) + k2
                                nc.vector.tensor_copy(
                                    scat[:, col, 0:PAYW], pscat[:])
                        USE_SCATTER = True
                        if USE_SCATTER:
                            nc.gpsimd.dma_scatter_add(
                                acc_d[:], scat[:],
                                sat_sb[:, sc * SCH * 4:(sc + 1) * SCH * 4],
                                SCH * MAXA, SCH * MAXA, ACCW,
                                single_packet=False)
                        else:
                            nc.sync.dma_start(
                                acc_d[0:SCH * MAXA, :].rearrange(
                                    "(k p) c -> p k c", p=128), scat[:])

            # ================= ReduceScatter =================
            USE_COLLECTIVE = True
            if USE_COLLECTIVE:
                nc.gpsimd.collective_compute(
                    "ReduceScatter", mybir.AluOpType.add,
                    ins=[acc_d[:]], outs=[rs_d[:]],
                    replica_groups=[list(range(NCORES))])
            else:
                nc.sync.dma_start(rs_d[:], acc_d[0:RSROWS, :])

            # ================= PHASE C =================
            with (
                tc.tile_pool(name="pc_sb", bufs=2) as pc,
                tc.tile_pool(name="pc_ps", bufs=2, space=MS.PSUM) as pcp,
            ):
                for ch in range(RSROWS // 128):
                    r0 = ch * 128
                    rsc = pc.tile([128, ACCW], BF16, tag="rsc")
                    nc.sync.dma_start(rsc[:], rs_d[r0:r0 + 128, :])
                    rscf = pc.tile([128, ACCW], F32, tag="rscf")
                    nc.vector.tensor_copy(rscf[:], rsc[:])
                    pT = pcp.tile([128, ACCW], F32, tag="pT")
                    for blk in range(3):
                        nc.tensor.transpose(
                            pT[:, blk * 128:(blk + 1) * 128],
                            rscf[:, blk * 128:(blk + 1) * 128], W["ident"][:])
                    p3i = pc.tile([C, 3, 128], F32, tag="p3i")
                    nc.sync.dma_start(p3i[:], p3ish_d[:, :, r0:r0 + 128])
                    p3n = pc.tile([C, 3, 128], F32, tag="p3n")
                    s3bc = pT[C:128, 0:128].rearrange(
                        "c n -> c () n").broadcast_to([C, 3, 128])
                    nc.vector.tensor_tensor(p3n[:], p3i[:], s3bc, AluOp.mult)
                    crossblks = [pT[0:C, 128:256], pT[C:128, 128:256], pT[0:C, 256:384]]
                    for x in range(3):
                        nc.vector.tensor_tensor(
                            p3n[:, x, :], p3n[:, x, :], crossblks[x], AluOp.add)
                    dsum = pc.tile([C, 128], F32, tag="dsum")
                    qx = pc.tile([C, 128], F32, tag="qx")
                    djs = pc.tile([C, 128], F32, tag="djs")
                    for x in range(3):
                        pdi = pcp.tile([C, 128], F32, tag="pdi")
                        pdj = pcp.tile([C, 128], F32, tag="pdj")
                        nc.tensor.matmul(pdi[:], W["Wdi"][:], p3n[:, x, :])
                        nc.tensor.matmul(pdj[:], W["Wdj"][:], p3n[:, x, :])
                        nc.scalar.activation(djs[:], pdj[:], Act.Copy)
                        if x == 0:
                            nc.vector.tensor_tensor(dsum[:], pdi[:], djs[:], AluOp.mult)
                        else:
                            nc.vector.tensor_tensor(qx[:], pdi[:], djs[:], AluOp.mult)
                            nc.vector.tensor_tensor(dsum[:], dsum[:], qx[:], AluOp.add)
                    p1t1c = pc.tile([C, 128], F32, tag="p1t1c")
                    nc.vector.tensor_tensor(p1t1c[:], dsum[:], pT[0:C, 0:128], AluOp.add)
                    nc.sync.dma_start(p1t1T_d[:, r0:r0 + 128], p1t1c[:])
                    p3t1c = pc.tile([C, 3, 128], F32, tag="p3t1c")
                    nc.vector.tensor_tensor(
                        p3t1c[:], p3n[:],
                        p1t1c[:].rearrange("c n -> c () n").broadcast_to([C, 3, 128]),
                        AluOp.mult)
                    nc.sync.dma_start(p3t1T_d[:, :, r0:r0 + 128], p3t1c[:])
    nc.compile()
    return nc


def _get_compiled():
    if "nc" not in _CACHE:
        _CACHE["nc"] = _build_nc()
    return _CACHE["nc"]


LAST_RESULT = None
LAST_RUN_S = None


def kernel(**inputs):
    global LAST_RESULT
    from concourse.bass_utils import run_bass_kernel_spmd
    in_maps = _preprocess(inputs)
    nc = _get_compiled()
    import time as _time
    _t0 = _time.time()
    LAST_RESULT = run_bass_kernel_spmd(nc, in_maps, list(range(NCORES)))
    res = LAST_RESULT.results
    global LAST_RUN_S
    LAST_RUN_S = _time.time() - _t0
    if not _USE_DEVICE_REDUCE:
        # host-side reduction + phase C fallback
        acc = np.zeros((NPAD, ACCW), np.float32)
        for c in range(NCORES):
            acc += np.asarray(res[c]["accout"], np.float32)
        t2s = acc[:N, :C]
        S3 = acc[:N, C:2 * C]
        cross = acc[:N, 2 * C:PAYW].reshape(N, 3, C)
        p1 = np.asarray(inputs["p1"], np.float32)
        p3 = np.asarray(inputs["p3"], np.float32)
        W3 = np.asarray(inputs["Wpp3a"], np.float32) @ np.asarray(
            inputs["Wpp3b"], np.float32)
        P3I = p3.reshape(-1, C) @ (W3 @ np.asarray(inputs["Wpxi"], np.float32))
        P3I = P3I.reshape(N, 3, C)
        p3n = P3I * S3[:, None, :] + cross
        di = p3n @ np.asarray(inputs["Wdi"], np.float32)
        dj = p3n @ np.asarray(inputs["Wdj"], np.float32)
        dot = (di * dj).sum(1)
        p1t1 = dot + t2s
        p3t1 = p3n * p1t1[:, None, :]
        return p1t1.astype(np.float32), p3t1.astype(np.float32)
    p1_parts = []
    p3_parts = []
    for c in range(NCORES):
        p1_parts.append(np.ascontiguousarray(res[c]["p1t1T"].T))
        p3_parts.append(np.ascontiguousarray(res[c]["p3t1T"].transpose(2, 1, 0)))
    p1t1 = np.concatenate(p1_parts, axis=0)[:N]
    p3t1 = np.concatenate(p3_parts, axis=0)[:N]
    return p1t1.astype(np.float32), p3t1.astype(np.float32)
